# revision 20
# baseline (speedup 1.0000x reference)
"""Trainium2 Bass kernel for nn_CausalAttention_5815385719336.

Dual-softmax attention: out = softmax(-QK^T/8) V Wo^T (+bias folds),
out_comp = softmax(+QK^T/8) V Wo^T.  B=2, S=2048, D=1024, H=16, DK=64.

Sharding (8 cores): Megatron-style head parallel.  Core c owns heads
(2c, 2c+1) = output dims [128c, 128c+128) of the QKV projections.  Each
core computes its head slice of Q/K/V for both batches, the full [S,S]
attention for its 4 (b, head) units (both softmax branches), and a
partial output projection o_slice @ Wo_slice^T.  The host sums the 8
partial outputs and adds the bias fold (bv @ Wo^T + bo).

v2 dataflow ("wide-exp"): the ACT engine is the roofline (268M exps
across branches).  Each activation instruction costs ~(N+352) cycles
for N free elements, so exps run at N=4096: score k-tiles [128,1024]
are staged PSUM->SBUF f32 by alternating DVE/Pool copies into a
[128,4,1024] ring, then ONE exp per branch covers 4 k-tiles.  PV
matmuls lag one 4-tile super-group behind the scores so the PE always
has ready work; projections for the next batch and output-projection
chunks drain from a background queue into the remaining PE slack.
Softmax denominators ride as a ones-column in the PV stationary (h0:
appended -> denom row 64; h1: prepended with the output window shifted
to partitions 63:128 -> denom row 63, data rows 64:128 landing exactly
where the output projection wants them - no partition-shift DMAs).
"""

import numpy as np
import ml_dtypes

B, S, D, H, DK = 2, 2048, 1024, 16, 64
NCORES = 8
HPC = H // NCORES          # heads per core = 2
DSL = HPC * DK             # d-slice per core = 128
P = 128
BF16 = ml_dtypes.bfloat16

_compiled = {}


def _install_drain_split():
    """walrus in this container rejects >1 sync wait on the Tile tail
    Drain; split extra waits into standalone wait_ge instructions."""
    import concourse.tile as tile
    from concourse.vector_clock import ScopedClock

    if getattr(tile.TileContext, "_drain_split_installed", False):
        return

    def _drain_and_barrier(self, tick_clock, wait_clock):
        nc = self.nc
        drain_inst = nc.sync.drain()
        wait_clock.add_sem_waits(
            drain_inst.ins, ScopedClock({None: tick_clock.global_clock})
        )
        si = drain_inst.ins.sync_info
        if si is not None and si.on_wait and len(si.on_wait) > 1:
            waits = list(si.on_wait)
            handles = {h.num: h for h in self.sems.allocated().values()}
            si.on_wait = waits[:1]
            for w in waits[1:]:
                assert w.wait_mode == "sem-ge-imm", w.wait_mode
                nc.sync.wait_ge(handles[w.id], w.wait_value)
        nc.all_engine_barrier()
        popped = nc._tile_sem_poison_stack.pop()
        assert popped is self._sem_poison
        nc.clear_and_free_semaphores(list(self.sems.allocated().values()))
        nc.all_engine_barrier()

    tile.TileContext._drain_and_barrier = _drain_and_barrier
    tile.TileContext._drain_split_installed = True


def _split_sync_waits(nc, max_waits=1):
    """walrus in this container has a small per-instruction sync-wait
    capacity.  Hoist excess waits onto standalone EventSemaphore
    instructions inserted just before the owner on the same engine —
    program order within an engine keeps the semantics identical."""
    from concourse import mybir

    n = 0
    for bb in nc.main_func.blocks:
        out = []
        for ins in bb.instructions:
            si = ins.sync_info
            if si is not None and si.on_wait and len(si.on_wait) > max_waits:
                waits = list(si.on_wait)
                for w in waits[:-max_waits]:
                    wi = mybir.InstEventSemaphore(name=f"W-split-{n}", ins=[], outs=[])
                    n += 1
                    wi.engine = ins.engine
                    wi.sync_info = mybir.SyncInfo(on_wait=[w], on_update=[])
                    out.append(wi)
                si.on_wait = waits[-max_waits:]
            out.append(ins)
        if n:
            bb.instructions = out


def _build():
    import concourse.bass as bass
    import concourse.tile as tile
    from concourse import mybir

    _install_drain_split()

    f32 = mybir.dt.float32
    bf16 = mybir.dt.bfloat16
    Exp = mybir.ActivationFunctionType.Exp
    Log = mybir.ActivationFunctionType.Ln
    NT = B * S                      # 4096 tokens
    ET = D // P                     # 8 e-tiles

    nc = bass.Bass()
    xt_d = nc.declare_dram_parameter("xt", [P, ET, NT], bf16, isOutput=False)
    wq_d = nc.declare_dram_parameter("wq", [P, ET, DSL], bf16, isOutput=False)
    wk_d = nc.declare_dram_parameter("wk", [P, ET, DSL], bf16, isOutput=False)
    wv_d = nc.declare_dram_parameter("wv", [P, ET, DSL], bf16, isOutput=False)
    wo_d = nc.declare_dram_parameter("wo", [P, D], bf16, isOutput=False)
    bq_d = nc.declare_dram_parameter("bq", [P, 1], f32, isOutput=False)
    bk_d = nc.declare_dram_parameter("bk", [P, 1], f32, isOutput=False)
    out_d = nc.declare_dram_parameter("out", [2, B, S, D], bf16, isOutput=True)

    KT = S // P                     # 16 k-tiles per batch
    TT = S // P                     # 16 token-tiles per batch
    QC = 2                          # q chunks per batch
    QW = S // QC                    # 1024
    G = 4                           # k-tiles per exp super-group
    NG = KT // G                    # super-groups per unit = 4

    with tile.TileContext(nc) as tc:
        with (
            tc.tile_pool(name="singles", bufs=1) as singles,
            tc.tile_pool(name="xst", bufs=2) as xst,
            tc.tile_pool(name="perb", bufs=2) as perb,
            tc.tile_pool(name="srp", bufs=2) as srp,
            tc.tile_pool(name="expp", bufs=2) as expp,
            tc.tile_pool(name="otsp", bufs=2) as otsp,
            tc.tile_pool(name="normp", bufs=2) as normp,
            tc.tile_pool(name="outp", bufs=3) as outp,
            # 8 PSUM banks: sc 1x[128,1024]=2, acc 2x[128,1024]=4,
            # small 2x[128,512]=2 (proj / outproj / bcast chunks).
            tc.tile_pool(name="ps_sc", bufs=1, space="PSUM") as ps_sc,
            tc.tile_pool(name="ps_acc", bufs=2, space="PSUM") as ps_acc,
            tc.tile_pool(name="ps_sm", bufs=2, space="PSUM") as ps_sm,
        ):
            wq = singles.tile([P, ET, DSL], bf16)
            nc.sync.dma_start(wq[:], wq_d[:])
            wk = singles.tile([P, ET, DSL], bf16)
            nc.sync.dma_start(wk[:], wk_d[:])
            wv = singles.tile([P, ET, DSL], bf16)
            nc.sync.dma_start(wv[:], wv_d[:])
            wo = singles.tile([P, D], bf16)
            nc.sync.dma_start(wo[:], wo_d[:])
            bq = singles.tile([P, 1], f32)
            nc.sync.dma_start(bq[:], bq_d[:])
            bk = singles.tile([P, 1], f32)
            nc.sync.dma_start(bk[:], bk_d[:])
            ones_sb = singles.tile([P, 64], bf16)
            nc.vector.memset(ones_sb[:], 1.0)

            # ---------- background queue ----------
            # bg_queue: PE-filler closures safe to drain now.
            # bg_delayed: closures whose deps were just emitted; promoted to
            # bg_queue at the next unit boundary so a drained chunk never
            # head-blocks the PE queue on an unfinished producer.
            bg_queue = []
            bg_delayed = []

            def drain_bg(n=1):
                for _ in range(n):
                    if not bg_queue:
                        return
                    bg_queue.pop(0)()

            # ---------- projections ----------
            def queue_projections(b):
                """Q/K/V projection chunk closures for batch b (streamed x).
                vt columns: [v_h0 (64) | ones | ones | v_h1 (64)]."""
                t0 = b * S
                qT = perb.tile([P, S], bf16, tag="qT", name=f"qT_{b}")
                kT = perb.tile([P, S], bf16, tag="kT", name=f"kT_{b}")
                vt = perb.tile([P, TT, 131], bf16, tag="vt", name=f"vt_{b}")
                nc.vector.memset(vt[:, :, 64], 1.0)
                nc.vector.memset(vt[:, :, 130], 1.0)
                cell = {}

                def load_chunk(qc4):
                    def go():
                        xtile = xst.tile([P, ET, 512], bf16, tag="xtile",
                                         name=f"xt_{b}_{qc4}")
                        nc.sync.dma_start(
                            xtile[:],
                            xt_d[:, :, t0 + qc4 * 512 : t0 + (qc4 + 1) * 512],
                        )
                        cell[qc4] = xtile
                    return go

                def qk_chunk(qc4, w_t, bias_t, dst):
                    def go():
                        xtile = cell[qc4]
                        ps = ps_sm.tile([P, 512], f32, tag="sm",
                                        name=f"pj_{b}_{qc4}_{id(w_t)}")
                        for et in range(ET):
                            nc.tensor.matmul(
                                ps, w_t[:, et, :], xtile[:, et, :],
                                start=(et == 0), stop=(et == ET - 1),
                            )
                        nc.vector.tensor_scalar_add(
                            dst[:, qc4 * 512 : (qc4 + 1) * 512], ps, bias_t
                        )
                    return go

                def v_chunk(qc4, vtt):
                    def go():
                        xtile = cell[qc4]
                        tt = qc4 * 4 + vtt
                        pv = ps_sm.tile([P, DSL], f32, tag="sm",
                                        name=f"pv_{b}_{tt}")
                        for et in range(ET):
                            nc.tensor.matmul(
                                pv, xtile[:, et, vtt * P : (vtt + 1) * P],
                                wv[:, et, :],
                                start=(et == 0), stop=(et == ET - 1),
                            )
                        nc.vector.tensor_copy(vt[:, tt, 0:64], pv[:, 0:64])
                        nc.vector.tensor_copy(vt[:, tt, 66:130],
                                              pv[:, 64:128])
                        # cols: 0:64 v_h0, 64 ones, 66:130 v_h1, 130 ones
                    return go

                loads = [load_chunk(c) for c in range(4)]
                ks = [qk_chunk(c, wk, bk, kT) for c in range(4)]
                qs = [qk_chunk(c, wq, bq, qT) for c in range(4)]
                vs = [[v_chunk(c, vtt) for vtt in range(4)] for c in range(4)]
                return qT, kT, vt, (loads, ks, qs, vs)

            # ---------- output projection ----------
            def outproj_chunks(b, oTs, qc):
                """Output projection chunk closures for token tiles of qc."""
                chunks = []

                def one(br, tt, oc):
                    def go():
                        po = ps_sm.tile([P, 512], f32, tag="sm",
                                        name=f"po_{b}_{br}_{tt}_{oc}")
                        nc.tensor.matmul(
                            po,
                            oTs[br][:, tt * P : (tt + 1) * P],
                            wo[:, oc * 512 : (oc + 1) * 512],
                            start=True,
                            stop=True,
                        )
                        ob = outp.tile([P, 512], bf16, tag="ob")
                        nc.vector.tensor_copy(ob[:], po[:])
                        nc.sync.dma_start(
                            out_d[br, b, tt * P : (tt + 1) * P,
                                  oc * 512 : (oc + 1) * 512], ob[:]
                        )
                    return go

                for br in range(2):
                    for tt in range(8 * qc, 8 * qc + 8):
                        for oc in range(2):
                            chunks.append(one(br, tt, oc))
                return chunks

            # ---------- normalize ----------
            def make_norm(b, h, qc, accP, accN, oTs, name):
                """Per unit: 1/denom via Ln+Exp on ACT (denom row 64 from
                the ones-column), broadcast via ones-matmul, multiply into
                the oTs slice.  h0 multiplies straight out of the PV
                accumulator; h1 routes through a bf16 copy + DMA shift to
                partitions 64:128 (PE out base must be 0/32/64-aligned)."""
                hp = 64 * h
                q0 = qc * QW

                def go():
                    for br, acc in ((0, accP), (1, accN)):
                        nm = f"{name}_{br}"
                        lnd = normp.tile([P, QW], f32, tag="lnd",
                                         name=f"lnd{nm}")
                        nc.scalar.activation(lnd[64:65, :], acc[64:65, :],
                                             Log)
                        rcp = normp.tile([P, QW], bf16, tag="rcp",
                                         name=f"rcp{nm}")
                        nc.scalar.activation(rcp[64:65, :], lnd[64:65, :],
                                             Exp, scale=-1.0)
                        oTu = normp.tile([P, QW], bf16, tag="oTu",
                                         name=f"oTu{nm}")
                        nc.scalar.copy(oTu[0:64, :], acc[0:64, :])
                        if h == 0:
                            src = oTu
                        else:
                            oTu2 = normp.tile([P, QW], bf16, tag="oTu2",
                                              name=f"oTu2{nm}")
                            nc.sync.dma_start(oTu2[64:128, :], oTu[0:64, :])
                            src = oTu2
                        for fh in range(2):
                            bc = ps_sm.tile([P, 512], f32, tag="sm",
                                            name=f"bc{nm}_{fh}")
                            nc.tensor.matmul(
                                bc[hp : hp + 64, :],
                                ones_sb[64:65, :],
                                rcp[64:65, fh * 512 : (fh + 1) * 512],
                                start=True,
                                stop=True,
                            )
                            nc.vector.tensor_mul(
                                oTs[br][hp : hp + 64,
                                        q0 + fh * 512 : q0 + (fh + 1) * 512],
                                src[hp : hp + 64,
                                    fh * 512 : (fh + 1) * 512],
                                bc[hp : hp + 64, :],
                            )
                return go

            # ---------- attention unit ----------
            def unit(b, h, qc, qT, kT, vt, oTs, pv_pending, prev_norm):
                """One (batch, head, q-chunk) unit.  PV matmuls lag one
                super-group; `pv_pending` carries the previous super's PV
                closures (possibly from the previous unit)."""
                hp = 64 * h
                vlo, vhi = (0, 65) if h == 0 else (66, 131)
                alo = 0
                q0 = qc * QW
                name = f"_{b}_{h}_{qc}"
                if bg_delayed:
                    bg_queue.extend(bg_delayed)
                    bg_delayed.clear()
                accP = ps_acc.tile([P, QW], f32, tag="acc",
                                   name=f"accP{name}")
                accN = ps_acc.tile([P, QW], f32, tag="acc",
                                   name=f"accN{name}")

                def make_pv(ea, eb, ktl, kt):
                    def go():
                        for acc, e in ((accP, ea), (accN, eb)):
                            for fh in range(2):
                                nc.tensor.matmul(
                                    acc[alo : alo + 65,
                                        fh * 512 : (fh + 1) * 512],
                                    vt[:, kt, vlo:vhi],
                                    e[:, ktl, fh * 512 : (fh + 1) * 512],
                                    start=(kt == 0),
                                    stop=(kt == KT - 1),
                                )
                    return go

                for g in range(NG):
                    sr = srp.tile([P, G, QW], f32, tag="sr",
                                  name=f"sr{name}_{g}")
                    for ktl in range(G):
                        kt = g * G + ktl
                        sc = ps_sc.tile([P, QW], f32, tag="sc",
                                        name=f"sc{name}_{kt}")
                        for fh in range(2):
                            nc.tensor.matmul(
                                sc[:, fh * 512 : (fh + 1) * 512],
                                kT[hp : hp + 64, kt * P : (kt + 1) * P],
                                qT[hp : hp + 64,
                                   q0 + fh * 512 : q0 + (fh + 1) * 512],
                                start=True,
                                stop=True,
                            )
                        nc.vector.tensor_copy(sr[:, ktl, :], sc[:])
                        if pv_pending:
                            pv_pending.pop(0)()
                        drain_bg(1)
                    if g == 1 and prev_norm is not None:
                        prev_norm()
                        prev_norm = None
                    ea = expp.tile([P, G, QW], bf16, tag="ea",
                                   name=f"ea{name}_{g}")
                    nc.scalar.activation(ea[:], sr[:], Exp, scale=-0.125)
                    eb = expp.tile([P, G, QW], bf16, tag="eb",
                                   name=f"eb{name}_{g}")
                    nc.scalar.activation(eb[:], sr[:], Exp, scale=0.125)
                    pv_pending.extend(
                        make_pv(ea, eb, ktl, g * G + ktl) for ktl in range(G)
                    )
                    drain_bg(2)
                if prev_norm is not None:
                    prev_norm()
                norm = make_norm(b, h, qc, accP, accN, oTs, name)
                return pv_pending, norm

            # ---------- emission ----------
            # prologue: just enough of batch-0's projections to start
            # attention (kT chunk 0, qT chunks 0+1); the rest drains as
            # background PE work inside the first units, ordered so each
            # chunk lands well before its consumer (k2 by super 2, v00..03
            # by the first PV drain in super 1, ...).
            qT0, kT0, vt0, (l0, k0, q0, v0) = queue_projections(0)
            for ch in (l0[0], l0[1], k0[0], q0[0], k0[1], q0[1]):
                ch()
            bg_queue.extend([l0[2]] + v0[0] + v0[1] +
                            [k0[2], q0[2], l0[3]] + v0[2] +
                            [k0[3], q0[3]] + v0[3])
            qT1, kT1, vt1, (l1, k1, q1, v1) = queue_projections(1)
            for c in range(4):        # batch-1 projections hide in b0 attn
                bg_queue.extend([l1[c], k1[c], q1[c]] + v1[c])

            oTs0 = [otsp.tile([P, S], bf16, tag=f"oTs{br}", name=f"oTs{br}_0")
                    for br in range(2)]
            oTs1 = [otsp.tile([P, S], bf16, tag=f"oTs{br}", name=f"oTs{br}_1")
                    for br in range(2)]

            pv, norm = [], None
            for b, qT, kT, vt, oTs in ((0, qT0, kT0, vt0, oTs0),
                                       (1, qT1, kT1, vt1, oTs1)):
                for qc in range(QC):
                    for h in range(HPC):
                        pv, norm = unit(b, h, qc, qT, kT, vt, oTs, pv, norm)

                    # outproj for this qc becomes available once BOTH heads'
                    # norms ran; the h=1 norm is still pending (it runs inside
                    # the next unit), so chain the enqueue onto it.
                    def qc_tail(b=b, qc=qc, oTs=oTs):
                        bg_delayed.extend(outproj_chunks(b, oTs, qc))
                    norm = _chain(norm, qc_tail)

            # tail: drain remaining PV, final norm, all remaining bg work
            while pv:
                pv.pop(0)()
                drain_bg(1)
            if norm is not None:
                norm()
            bg_queue.extend(bg_delayed)
            bg_delayed.clear()
            while bg_queue:
                drain_bg(1)
    _split_sync_waits(nc)
    return nc


def _chain(f, g):
    def go():
        f()
        g()
    return go


def _get_nc():
    if "nc" not in _compiled:
        _compiled["nc"] = _build()
    return _compiled["nc"]


def _prep_in_maps(x, Wq, bq, Wk, bk, Wv, bv, Wo, bo):
    ET = D // P
    xf = np.ascontiguousarray(x.reshape(B * S, D))
    # x^T tiled: [p, et, token], e = et*128 + p
    xt = np.ascontiguousarray(
        xf.T.reshape(ET, P, B * S).transpose(1, 0, 2)
    ).astype(BF16)
    in_maps = []
    for c in range(NCORES):
        sl = slice(DSL * c, DSL * (c + 1))
        wqt = np.ascontiguousarray(
            Wq[sl].T.reshape(ET, P, DSL).transpose(1, 0, 2)
        ).astype(BF16)
        wkt = np.ascontiguousarray(
            Wk[sl].T.reshape(ET, P, DSL).transpose(1, 0, 2)
        ).astype(BF16)
        wvt = np.ascontiguousarray(
            Wv[sl].T.reshape(ET, P, DSL).transpose(1, 0, 2)
        ).astype(BF16)
        wot = np.ascontiguousarray(Wo[:, sl].T).astype(BF16)
        in_maps.append(
            {
                "xt": xt,
                "wq": wqt,
                "wk": wkt,
                "wv": wvt,
                "wo": wot,
                "bq": np.ascontiguousarray(bq[sl].reshape(P, 1)).astype(np.float32),
                "bk": np.ascontiguousarray(bk[sl].reshape(P, 1)).astype(np.float32),
            }
        )
    return in_maps


def kernel(x, Wq, bq, Wk, bk, Wv, bv, Wo, bo, _trace=False, _tmpdir=None):
    from concourse.bass_utils import run_bass_kernel_spmd

    x, Wq, bq, Wk, bk, Wv, bv, Wo, bo = (
        np.asarray(a, dtype=np.float32)
        for a in (x, Wq, bq, Wk, bk, Wv, bv, Wo, bo)
    )
    nc = _get_nc()
    in_maps = _prep_in_maps(x, Wq, bq, Wk, bk, Wv, bv, Wo, bo)
    res = run_bass_kernel_spmd(
        nc, in_maps, core_ids=list(range(NCORES)), trace=_trace, tmpdir=_tmpdir
    )
    total = np.zeros((2, B, S, D), np.float32)
    for c in range(NCORES):
        total += np.asarray(res.results[c]["out"], dtype=np.float32)
    const_vec = (bv @ Wo.T + bo).astype(np.float32)
    out = total[0] + const_vec
    out_comp = total[1] + const_vec
    if _trace:
        kernel._last_result = res
    return (out, out_comp)


# revision 22
# speedup vs baseline: 1.0716x; 1.0716x over previous
"""Trainium2 Bass kernel for nn_CausalAttention_5815385719336.

Dual-softmax attention: out = softmax(-QK^T/8) V Wo^T (+bias folds),
out_comp = softmax(+QK^T/8) V Wo^T.  B=2, S=2048, D=1024, H=16, DK=64.

Sharding (8 cores): Megatron-style head parallel.  Core c owns heads
(2c, 2c+1) = output dims [128c, 128c+128) of the QKV projections.  Each
core computes its head slice of Q/K/V for both batches, the full [S,S]
attention for its 4 (b, head) units (both softmax branches), and a
partial output projection o_slice @ Wo_slice^T.  The host sums the 8
partial outputs and adds the bias fold (bv @ Wo^T + bo).

v2 dataflow ("wide-exp"): the ACT engine is the roofline (268M exps
across branches).  Each activation instruction costs ~(N+352) cycles
for N free elements, so exps run at N=4096: score k-tiles [128,1024]
are staged PSUM->SBUF f32 by alternating DVE/Pool copies into a
[128,4,1024] ring, then ONE exp per branch covers 4 k-tiles.  PV
matmuls lag one 4-tile super-group behind the scores so the PE always
has ready work; projections for the next batch and output-projection
chunks drain from a background queue into the remaining PE slack.
Softmax denominators ride as a ones-column in the PV stationary (h0:
appended -> denom row 64; h1: prepended with the output window shifted
to partitions 63:128 -> denom row 63, data rows 64:128 landing exactly
where the output projection wants them - no partition-shift DMAs).
"""

import numpy as np
import ml_dtypes

B, S, D, H, DK = 2, 2048, 1024, 16, 64
NCORES = 8
HPC = H // NCORES          # heads per core = 2
DSL = HPC * DK             # d-slice per core = 128
P = 128
BF16 = ml_dtypes.bfloat16

_compiled = {}


def _install_drain_split():
    """walrus in this container rejects >1 sync wait on the Tile tail
    Drain; split extra waits into standalone wait_ge instructions."""
    import concourse.tile as tile
    from concourse.vector_clock import ScopedClock

    if getattr(tile.TileContext, "_drain_split_installed", False):
        return

    def _drain_and_barrier(self, tick_clock, wait_clock):
        nc = self.nc
        drain_inst = nc.sync.drain()
        wait_clock.add_sem_waits(
            drain_inst.ins, ScopedClock({None: tick_clock.global_clock})
        )
        si = drain_inst.ins.sync_info
        if si is not None and si.on_wait and len(si.on_wait) > 1:
            waits = list(si.on_wait)
            handles = {h.num: h for h in self.sems.allocated().values()}
            si.on_wait = waits[:1]
            for w in waits[1:]:
                assert w.wait_mode == "sem-ge-imm", w.wait_mode
                nc.sync.wait_ge(handles[w.id], w.wait_value)
        nc.all_engine_barrier()
        popped = nc._tile_sem_poison_stack.pop()
        assert popped is self._sem_poison
        nc.clear_and_free_semaphores(list(self.sems.allocated().values()))
        nc.all_engine_barrier()

    tile.TileContext._drain_and_barrier = _drain_and_barrier
    tile.TileContext._drain_split_installed = True


def _split_sync_waits(nc, max_waits=1):
    """walrus in this container has a small per-instruction sync-wait
    capacity.  Hoist excess waits onto standalone EventSemaphore
    instructions inserted just before the owner on the same engine —
    program order within an engine keeps the semantics identical."""
    from concourse import mybir

    n = 0
    for bb in nc.main_func.blocks:
        out = []
        for ins in bb.instructions:
            si = ins.sync_info
            if si is not None and si.on_wait and len(si.on_wait) > max_waits:
                waits = list(si.on_wait)
                for w in waits[:-max_waits]:
                    wi = mybir.InstEventSemaphore(name=f"W-split-{n}", ins=[], outs=[])
                    n += 1
                    wi.engine = ins.engine
                    wi.sync_info = mybir.SyncInfo(on_wait=[w], on_update=[])
                    out.append(wi)
                si.on_wait = waits[-max_waits:]
            out.append(ins)
        if n:
            bb.instructions = out


def _build():
    import concourse.bass as bass
    import concourse.tile as tile
    from concourse import mybir

    _install_drain_split()

    f32 = mybir.dt.float32
    bf16 = mybir.dt.bfloat16
    Exp = mybir.ActivationFunctionType.Exp
    Log = mybir.ActivationFunctionType.Ln
    NT = B * S                      # 4096 tokens
    ET = D // P                     # 8 e-tiles

    nc = bass.Bass()
    xt_d = nc.declare_dram_parameter("xt", [P, ET, NT], bf16, isOutput=False)
    wq_d = nc.declare_dram_parameter("wq", [P, ET, DSL], bf16, isOutput=False)
    wk_d = nc.declare_dram_parameter("wk", [P, ET, DSL], bf16, isOutput=False)
    wv_d = nc.declare_dram_parameter("wv", [P, ET, DSL], bf16, isOutput=False)
    wo_d = nc.declare_dram_parameter("wo", [P, D], bf16, isOutput=False)
    bq_d = nc.declare_dram_parameter("bq", [P, 1], f32, isOutput=False)
    bk_d = nc.declare_dram_parameter("bk", [P, 1], f32, isOutput=False)
    out_d = nc.declare_dram_parameter("out", [2, B, S, D], bf16, isOutput=True)

    KT = S // P                     # 16 k-tiles per batch
    TT = S // P                     # 16 token-tiles per batch
    QC = 2                          # q chunks per batch
    QW = S // QC                    # 1024
    G = 4                           # k-tiles per exp super-group
    NG = KT // G                    # super-groups per unit = 4

    with tile.TileContext(nc) as tc:
        with (
            tc.tile_pool(name="singles", bufs=1) as singles,
            tc.tile_pool(name="xst", bufs=2) as xst,
            tc.tile_pool(name="perb", bufs=2) as perb,
            tc.tile_pool(name="srp", bufs=2) as srp,
            tc.tile_pool(name="expp", bufs=2) as expp,
            tc.tile_pool(name="otsp", bufs=2) as otsp,
            tc.tile_pool(name="normp", bufs=2) as normp,
            tc.tile_pool(name="outp", bufs=3) as outp,
            # 8 PSUM banks: sc 1x[128,1024]=2, acc 2x[128,1024]=4,
            # small 2x[128,512]=2 (proj / outproj / bcast chunks).
            tc.tile_pool(name="ps_sc", bufs=1, space="PSUM") as ps_sc,
            tc.tile_pool(name="ps_acc", bufs=2, space="PSUM") as ps_acc,
            tc.tile_pool(name="ps_sm", bufs=2, space="PSUM") as ps_sm,
        ):
            wq = singles.tile([P, ET, DSL], bf16)
            nc.sync.dma_start(wq[:], wq_d[:])
            wk = singles.tile([P, ET, DSL], bf16)
            nc.sync.dma_start(wk[:], wk_d[:])
            wv = singles.tile([P, ET, DSL], bf16)
            nc.sync.dma_start(wv[:], wv_d[:])
            wo = singles.tile([P, D], bf16)
            nc.sync.dma_start(wo[:], wo_d[:])
            bq = singles.tile([P, 1], f32)
            nc.sync.dma_start(bq[:], bq_d[:])
            bk = singles.tile([P, 1], f32)
            nc.sync.dma_start(bk[:], bk_d[:])
            ones_sb = singles.tile([P, 64], bf16)
            nc.vector.memset(ones_sb[:], 1.0)

            # ---------- background queue ----------
            # bg_queue: PE-filler closures safe to drain now.
            # bg_delayed: closures whose deps were just emitted; promoted to
            # bg_queue at the next unit boundary so a drained chunk never
            # head-blocks the PE queue on an unfinished producer.
            bg_queue = []
            bg_delayed = []

            def drain_bg(n=1):
                for _ in range(n):
                    if not bg_queue:
                        return
                    bg_queue.pop(0)()

            # ---------- projections ----------
            def queue_projections(b):
                """Q/K/V projection chunk closures for batch b (streamed x).
                vt columns: [v_h0 (64) | ones | ones | v_h1 (64)]."""
                t0 = b * S
                qT = perb.tile([P, S], bf16, tag="qT", name=f"qT_{b}")
                kT = perb.tile([P, S], bf16, tag="kT", name=f"kT_{b}")
                vt = perb.tile([P, TT, 131], bf16, tag="vt", name=f"vt_{b}")
                nc.vector.memset(vt[:, :, 64], 1.0)
                nc.vector.memset(vt[:, :, 130], 1.0)
                cell = {}

                def load_chunk(qc4):
                    def go():
                        xtile = xst.tile([P, ET, 512], bf16, tag="xtile",
                                         name=f"xt_{b}_{qc4}")
                        nc.sync.dma_start(
                            xtile[:],
                            xt_d[:, :, t0 + qc4 * 512 : t0 + (qc4 + 1) * 512],
                        )
                        cell[qc4] = xtile
                    return go

                def qk_chunk(qc4, w_t, bias_t, dst):
                    def go():
                        xtile = cell[qc4]
                        ps = ps_sm.tile([P, 512], f32, tag="sm",
                                        name=f"pj_{b}_{qc4}_{id(w_t)}")
                        for et in range(ET):
                            nc.tensor.matmul(
                                ps, w_t[:, et, :], xtile[:, et, :],
                                start=(et == 0), stop=(et == ET - 1),
                            )
                        nc.vector.tensor_scalar_add(
                            dst[:, qc4 * 512 : (qc4 + 1) * 512], ps, bias_t
                        )
                    return go

                def v_chunk(qc4, vtt):
                    def go():
                        xtile = cell[qc4]
                        tt = qc4 * 4 + vtt
                        pv = ps_sm.tile([P, DSL], f32, tag="sm",
                                        name=f"pv_{b}_{tt}")
                        for et in range(ET):
                            nc.tensor.matmul(
                                pv, xtile[:, et, vtt * P : (vtt + 1) * P],
                                wv[:, et, :],
                                start=(et == 0), stop=(et == ET - 1),
                            )
                        nc.vector.tensor_copy(vt[:, tt, 0:64], pv[:, 0:64])
                        nc.vector.tensor_copy(vt[:, tt, 66:130],
                                              pv[:, 64:128])
                        # cols: 0:64 v_h0, 64 ones, 66:130 v_h1, 130 ones
                    return go

                loads = [load_chunk(c) for c in range(4)]
                ks = [qk_chunk(c, wk, bk, kT) for c in range(4)]
                qs = [qk_chunk(c, wq, bq, qT) for c in range(4)]
                vs = [[v_chunk(c, vtt) for vtt in range(4)] for c in range(4)]
                return qT, kT, vt, (loads, ks, qs, vs)

            # ---------- output projection ----------
            def outproj_chunks(b, oTs, qc):
                """Output projection chunk closures for token tiles of qc.
                Near the kernel tail the DVE is the pacer, so the last
                batch routes a share of the PSUM->SBUF casts to ACT."""
                chunks = []

                def one(br, tt, oc, on_act):
                    def go():
                        po = ps_sm.tile([P, 512], f32, tag="sm",
                                        name=f"po_{b}_{br}_{tt}_{oc}")
                        nc.tensor.matmul(
                            po,
                            oTs[br][:, tt * P : (tt + 1) * P],
                            wo[:, oc * 512 : (oc + 1) * 512],
                            start=True,
                            stop=True,
                        )
                        ob = outp.tile([P, 512], bf16, tag="ob")
                        if on_act:
                            nc.scalar.copy(ob[:], po[:])
                        else:
                            nc.vector.tensor_copy(ob[:], po[:])
                        nc.sync.dma_start(
                            out_d[br, b, tt * P : (tt + 1) * P,
                                  oc * 512 : (oc + 1) * 512], ob[:]
                        )
                    return go

                i = 0
                for br in range(2):
                    for tt in range(8 * qc, 8 * qc + 8):
                        for oc in range(2):
                            if b == 1 and qc == 1:
                                on_act = i % 2 == 1
                            elif b == 1:
                                on_act = i % 4 == 3
                            else:
                                on_act = False
                            chunks.append(one(br, tt, oc, on_act))
                            i += 1
                return chunks

            # ---------- normalize ----------
            def make_norm(b, h, qc, accP, accN, oTs, name):
                """Per unit: 1/denom via Ln+Exp on ACT (denom row 64 from
                the ones-column), broadcast via ones-matmul, multiply into
                the oTs slice.  h0 multiplies straight out of the PV
                accumulator; h1 routes through a bf16 copy + DMA shift to
                partitions 64:128 (PE out base must be 0/32/64-aligned)."""
                hp = 64 * h
                q0 = qc * QW

                def go():
                    for br, acc in ((0, accP), (1, accN)):
                        nm = f"{name}_{br}"
                        lnd = normp.tile([P, QW], f32, tag="lnd",
                                         name=f"lnd{nm}")
                        nc.scalar.activation(lnd[64:65, :], acc[64:65, :],
                                             Log)
                        rcp = normp.tile([P, QW], bf16, tag="rcp",
                                         name=f"rcp{nm}")
                        nc.scalar.activation(rcp[64:65, :], lnd[64:65, :],
                                             Exp, scale=-1.0)
                        oTu = normp.tile([P, QW], bf16, tag="oTu",
                                         name=f"oTu{nm}")
                        nc.scalar.copy(oTu[0:64, :], acc[0:64, :])
                        if h == 0:
                            src = oTu
                        else:
                            oTu2 = normp.tile([P, QW], bf16, tag="oTu2",
                                              name=f"oTu2{nm}")
                            nc.sync.dma_start(oTu2[64:128, :], oTu[0:64, :])
                            src = oTu2
                        for fh in range(2):
                            bc = ps_sm.tile([P, 512], f32, tag="sm",
                                            name=f"bc{nm}_{fh}")
                            nc.tensor.matmul(
                                bc[hp : hp + 64, :],
                                ones_sb[64:65, :],
                                rcp[64:65, fh * 512 : (fh + 1) * 512],
                                start=True,
                                stop=True,
                            )
                            nc.vector.tensor_mul(
                                oTs[br][hp : hp + 64,
                                        q0 + fh * 512 : q0 + (fh + 1) * 512],
                                src[hp : hp + 64,
                                    fh * 512 : (fh + 1) * 512],
                                bc[hp : hp + 64, :],
                            )
                return go

            # ---------- attention unit ----------
            def unit(b, h, qc, qT, kT, vt, oTs, pv_pending, prev_norm):
                """One (batch, head, q-chunk) unit.  PV matmuls lag one
                super-group; `pv_pending` carries the previous super's PV
                closures (possibly from the previous unit)."""
                hp = 64 * h
                vlo, vhi = (0, 65) if h == 0 else (66, 131)
                alo = 0
                q0 = qc * QW
                name = f"_{b}_{h}_{qc}"
                if bg_delayed:
                    bg_queue.extend(bg_delayed)
                    bg_delayed.clear()
                accP = ps_acc.tile([P, QW], f32, tag="acc",
                                   name=f"accP{name}")
                accN = ps_acc.tile([P, QW], f32, tag="acc",
                                   name=f"accN{name}")

                def make_pv(ea, eb, ktl, kt):
                    def go():
                        for acc, e in ((accP, ea), (accN, eb)):
                            for fh in range(2):
                                nc.tensor.matmul(
                                    acc[alo : alo + 65,
                                        fh * 512 : (fh + 1) * 512],
                                    vt[:, kt, vlo:vhi],
                                    e[:, ktl, fh * 512 : (fh + 1) * 512],
                                    start=(kt == 0),
                                    stop=(kt == KT - 1),
                                )
                    return go

                for g in range(NG):
                    # norm of the previous unit: its accumulators are
                    # complete once that unit's g3 PV drained (our g1), and
                    # it must precede this unit's first PV drain (g2) for
                    # the ps_acc WAR rotation.
                    if g == 2 and prev_norm is not None:
                        prev_norm()
                        prev_norm = None
                    sr = srp.tile([P, G, QW], f32, tag="sr",
                                  name=f"sr{name}_{g}")
                    for ktl in range(G):
                        kt = g * G + ktl
                        sc = ps_sc.tile([P, QW], f32, tag="sc",
                                        name=f"sc{name}_{kt}")
                        for fh in range(2):
                            nc.tensor.matmul(
                                sc[:, fh * 512 : (fh + 1) * 512],
                                kT[hp : hp + 64, kt * P : (kt + 1) * P],
                                qT[hp : hp + 64,
                                   q0 + fh * 512 : q0 + (fh + 1) * 512],
                                start=True,
                                stop=True,
                            )
                        nc.vector.tensor_copy(sr[:, ktl, :], sc[:])
                        # two-super PV lag: drain only above depth 4 so a
                        # super's PV never chases its own exps through the
                        # PE FIFO (that stall re-throttles the HAM clock)
                        if len(pv_pending) > 4:
                            pv_pending.pop(0)()
                        drain_bg(1)
                    ea = expp.tile([P, G, QW], bf16, tag="ea", bufs=3,
                                   name=f"ea{name}_{g}")
                    nc.scalar.activation(ea[:], sr[:], Exp, scale=-0.125)
                    eb = expp.tile([P, G, QW], bf16, tag="eb", bufs=3,
                                   name=f"eb{name}_{g}")
                    nc.scalar.activation(eb[:], sr[:], Exp, scale=0.125)
                    pv_pending.extend(
                        make_pv(ea, eb, ktl, g * G + ktl) for ktl in range(G)
                    )
                    drain_bg(2)
                if prev_norm is not None:
                    prev_norm()
                norm = make_norm(b, h, qc, accP, accN, oTs, name)
                return pv_pending, norm

            # ---------- emission ----------
            # prologue: just enough of batch-0's projections to start
            # attention (kT chunk 0, qT chunks 0+1); the rest drains as
            # background PE work inside the first units, ordered so each
            # chunk lands well before its consumer (k2 by super 2, v00..03
            # by the first PV drain in super 1, ...).
            qT0, kT0, vt0, (l0, k0, q0, v0) = queue_projections(0)
            for ch in (l0[0], l0[1], k0[0], q0[0], k0[1], q0[1]):
                ch()
            bg_queue.extend([l0[2]] + v0[0] + v0[1] +
                            [k0[2], q0[2], l0[3]] + v0[2] +
                            [k0[3], q0[3]] + v0[3])
            qT1, kT1, vt1, (l1, k1, q1, v1) = queue_projections(1)
            for c in range(4):        # batch-1 projections hide in b0 attn
                bg_queue.extend([l1[c], k1[c], q1[c]] + v1[c])

            oTs0 = [otsp.tile([P, S], bf16, tag=f"oTs{br}", name=f"oTs{br}_0")
                    for br in range(2)]
            oTs1 = [otsp.tile([P, S], bf16, tag=f"oTs{br}", name=f"oTs{br}_1")
                    for br in range(2)]

            pv, norm = [], None
            for b, qT, kT, vt, oTs in ((0, qT0, kT0, vt0, oTs0),
                                       (1, qT1, kT1, vt1, oTs1)):
                for qc in range(QC):
                    for h in range(HPC):
                        pv, norm = unit(b, h, qc, qT, kT, vt, oTs, pv, norm)

                    # outproj for this qc becomes available once BOTH heads'
                    # norms ran; the h=1 norm is still pending (it runs inside
                    # the next unit), so chain the enqueue onto it.
                    def qc_tail(b=b, qc=qc, oTs=oTs):
                        bg_delayed.extend(outproj_chunks(b, oTs, qc))
                    norm = _chain(norm, qc_tail)

            # tail: drain remaining PV, final norm, all remaining bg work
            while pv:
                pv.pop(0)()
                drain_bg(1)
            if norm is not None:
                norm()
            bg_queue.extend(bg_delayed)
            bg_delayed.clear()
            while bg_queue:
                drain_bg(1)
    _split_sync_waits(nc)
    return nc


def _chain(f, g):
    def go():
        f()
        g()
    return go


def _get_nc():
    if "nc" not in _compiled:
        _compiled["nc"] = _build()
    return _compiled["nc"]


def _prep_in_maps(x, Wq, bq, Wk, bk, Wv, bv, Wo, bo):
    ET = D // P
    xf = np.ascontiguousarray(x.reshape(B * S, D))
    # x^T tiled: [p, et, token], e = et*128 + p
    xt = np.ascontiguousarray(
        xf.T.reshape(ET, P, B * S).transpose(1, 0, 2)
    ).astype(BF16)
    in_maps = []
    for c in range(NCORES):
        sl = slice(DSL * c, DSL * (c + 1))
        wqt = np.ascontiguousarray(
            Wq[sl].T.reshape(ET, P, DSL).transpose(1, 0, 2)
        ).astype(BF16)
        wkt = np.ascontiguousarray(
            Wk[sl].T.reshape(ET, P, DSL).transpose(1, 0, 2)
        ).astype(BF16)
        wvt = np.ascontiguousarray(
            Wv[sl].T.reshape(ET, P, DSL).transpose(1, 0, 2)
        ).astype(BF16)
        wot = np.ascontiguousarray(Wo[:, sl].T).astype(BF16)
        in_maps.append(
            {
                "xt": xt,
                "wq": wqt,
                "wk": wkt,
                "wv": wvt,
                "wo": wot,
                "bq": np.ascontiguousarray(bq[sl].reshape(P, 1)).astype(np.float32),
                "bk": np.ascontiguousarray(bk[sl].reshape(P, 1)).astype(np.float32),
            }
        )
    return in_maps


def kernel(x, Wq, bq, Wk, bk, Wv, bv, Wo, bo, _trace=False, _tmpdir=None):
    from concourse.bass_utils import run_bass_kernel_spmd

    x, Wq, bq, Wk, bk, Wv, bv, Wo, bo = (
        np.asarray(a, dtype=np.float32)
        for a in (x, Wq, bq, Wk, bk, Wv, bv, Wo, bo)
    )
    nc = _get_nc()
    in_maps = _prep_in_maps(x, Wq, bq, Wk, bk, Wv, bv, Wo, bo)
    res = run_bass_kernel_spmd(
        nc, in_maps, core_ids=list(range(NCORES)), trace=_trace, tmpdir=_tmpdir
    )
    total = np.zeros((2, B, S, D), np.float32)
    for c in range(NCORES):
        total += np.asarray(res.results[c]["out"], dtype=np.float32)
    const_vec = (bv @ Wo.T + bo).astype(np.float32)
    out = total[0] + const_vec
    out_comp = total[1] + const_vec
    if _trace:
        kernel._last_result = res
    return (out, out_comp)
